# revision 15
# baseline (speedup 1.0000x reference)
"""int4 weight-only quantized GEMV on 8 TRN2 NeuronCores — TensorE formulation.

out[1, n] = sum_k A[1, k] * W[n, k],   W = dequant(B packed nibbles, scales/zeros)
A: [1, 8192] fp16, B: [16384, 4096] int32 (one byte per elem, 2 nibbles),
scalesAndZeros: [16384, 256, 2] fp16 (group=32 along K).

Sharding: N=16384 rows split across 8 cores (2048 rows each); A replicated.

Key idea: upload B as dense BYTES, TRANSPOSED to [KH=4096, NS=2048] so the
contraction (k) direction lies on SBUF partitions. Then the per-group dot
    dotg[g, n] = sum_{k in g} nib[n, k] * A[k]
is a TensorE matmul with a block-diagonal stationary built from A:
    byte b = lo + 16*hi  =>  lo*Ae + hi*Ao = b*Ae + hi*(Ao - 16*Ae)
    stream1 = bytes as fp16   (ScalarE activation-copy cast)
    stream2 = b >> 4          (DVE tensor_scalar, 4x mode)
Each 128-partition chunk covers 8 K-groups -> psum rows [8, n]. 32 chunks fill
psum dotg [256 groups, 2048 n] (2 tiles x 4 n-quarters, 8 banks).

Endgame: out[n] = sum_g sT[g,n]*dotg[g,n] + zsum[n], where
zsum[n] = sum_g (z - 8 s)[n,g]*sumA[g] is folded on the host (O(N*K/32)).
sT multiply on DVE; the partition-direction g-sum is a ones-vector matmul.

Host-side prep is layout-only/O(N*K/32) arithmetic: byte view of B (no
unpacking of nibbles on host), per-core transposes, W1/W2/zsum tables.
"""

import numpy as np

import concourse.bass as bass
import concourse.bacc as bacc
import concourse.mybir as mybir
from concourse import tile
from concourse.bass_utils import run_bass_kernel_spmd

FP16 = mybir.dt.float16
FP32 = mybir.dt.float32
UINT8 = mybir.dt.uint8
INT16 = mybir.dt.int16
Alu = mybir.AluOpType
Act = mybir.ActivationFunctionType

M, K, N = 1, 8192, 16384
KH = K // 2          # 4096 packed bytes per row
GROUP = 32
NG = K // GROUP      # 256 groups
NCORES = 8
NS = N // NCORES     # 2048 rows per core
P = 128              # partitions
NCH = KH // P        # 32 K-chunks per core
GPC = P // (GROUP // 2)   # 8 groups per chunk
MMF = 512            # matmul free-dim tile (one PSUM bank of fp32)
NQ = NS // MMF       # 4 n-quarters


def build_program(ns=NS):
    nq = ns // MMF
    nc = bacc.Bacc()
    bt_d = nc.declare_dram_parameter("BT", [KH, ns], UINT8, isOutput=False)
    st_d = nc.declare_dram_parameter("ST", [NG, ns], FP16, isOutput=False)
    w1_d = nc.declare_dram_parameter("W1", [P, P * NCH], FP16, isOutput=False)
    w2_d = nc.declare_dram_parameter("W2", [P, P * NCH], FP16, isOutput=False)
    on_d = nc.declare_dram_parameter("ONES", [P, 1], FP16, isOutput=False)
    zs_d = nc.declare_dram_parameter("ZS", [1, ns], FP16, isOutput=False)
    cg_d = nc.declare_dram_parameter("CG", [P, 2], FP32, isOutput=False)
    out_d = nc.declare_dram_parameter("OUT", [ns], FP16, isOutput=True)

    with tile.TileContext(nc) as tc:
        with (
            tc.tile_pool(name="const", bufs=1) as cpool,
            tc.tile_pool(name="bin", bufs=3) as bpool,
            tc.tile_pool(name="work", bufs=3) as wpool,
            tc.tile_pool(name="small", bufs=2) as spool,
            tc.tile_pool(name="keep", bufs=1) as kpool,
            tc.tile_pool(name="psum", bufs=2, space="PSUM") as ppool,
        ):
            w1 = cpool.tile([P, P * NCH], FP16)
            nc.sync.dma_start(out=w1[:, :], in_=w1_d[:, :])
            w2 = cpool.tile([P, P * NCH], FP16)
            nc.sync.dma_start(out=w2[:, :], in_=w2_d[:, :])
            ones = cpool.tile([P, 1], FP16)
            nc.sync.dma_start(out=ones[:, :], in_=on_d[:, :])
            zs = cpool.tile([1, ns], FP16)
            nc.sync.dma_start(out=zs[:, :], in_=zs_d[:, :])
            cg = cpool.tile([P, 2], FP32)
            nc.sync.dma_start(out=cg[:, :], in_=cg_d[:, :])
            st = []
            for t in range(2):
                s = cpool.tile([P, ns], FP16)
                nc.sync.dma_start(out=s[:, :], in_=st_d[t * P : (t + 1) * P, :])
                st.append(s)

            sp_tiles = {}
            for t in range(2):
                D = [
                    ppool.tile([P, MMF], FP32, tag=f"ps{q}", name=f"D{t}{q}")
                    for q in range(nq)
                ]
                for r in range(NCH // 2):
                    c = (NCH // 2) * t + r
                    xt = bpool.tile([P, ns], INT16, tag="xt")
                    nc.gpsimd.dma_start(out=xt[:, :], in_=bt_d[c * P : (c + 1) * P, :])
                    # bytes were biased by -128 on host so int8 holds them;
                    # undo the bias during each extraction.
                    bf = wpool.tile([P, ns], FP16, tag="bf")
                    nc.scalar.activation(out=bf[:, :], in_=xt[:, :], func=Act.Copy)
                    # bitVec ops cannot cast, so shift to int16 then cast-copy.
                    hi_i = wpool.tile([P, ns], INT16, tag="hi_i")
                    nc.vector.tensor_scalar(
                        out=hi_i[:, :], in0=xt[:, :], scalar1=4, scalar2=None,
                        op0=Alu.logical_shift_right,
                    )
                    hi = wpool.tile([P, ns], FP16, tag="hi")
                    nc.vector.tensor_copy(out=hi[:, :], in_=hi_i[:, :])
                    ws = slice(P * c, P * (c + 1))
                    for q in range(nq):
                        qs = slice(MMF * q, MMF * (q + 1))
                        nc.tensor.matmul(
                            D[q][:, :], w1[:, ws], bf[:, qs],
                            start=(r == 0), stop=False,
                        )
                        nc.tensor.matmul(
                            D[q][:, :], w2[:, ws], hi[:, qs],
                            start=False, stop=(r == NCH // 2 - 1),
                        )
                for q in range(nq):
                    qs = slice(MMF * q, MMF * (q + 1))
                    d = spool.tile([P, MMF], FP16, tag="d")
                    nc.scalar.activation(
                        out=d[:, :], in_=D[q][:, :], func=Act.Identity,
                        bias=cg[:, t : t + 1],
                    )
                    sp = kpool.tile([P, MMF], FP16, tag=f"sp{t}{q}")
                    nc.vector.tensor_tensor(
                        out=sp[:, :], in0=d[:, :], in1=st[t][:, qs], op=Alu.mult
                    )
                    sp_tiles[(t, q)] = sp

            fin = kpool.tile([1, ns], FP16)
            for q in range(nq):
                qs = slice(MMF * q, MMF * (q + 1))
                O = ppool.tile([P, MMF], FP32, tag=f"ps{q}")
                nc.tensor.matmul(
                    O[0:1, :], ones[:, :], sp_tiles[(0, q)][:, :],
                    start=True, stop=False,
                )
                nc.tensor.matmul(
                    O[0:1, :], ones[:, :], sp_tiles[(1, q)][:, :],
                    start=False, stop=True,
                )
                nc.vector.tensor_tensor(
                    out=fin[0:1, qs], in0=O[0:1, :], in1=zs[0:1, qs], op=Alu.add
                )
            nc.sync.dma_start(out=out_d[:], in_=fin[0:1, :])
    nc.finalize()
    return nc


_NC_CACHE = {}


def _get_program(ns=NS):
    if ns not in _NC_CACHE:
        _NC_CACHE[ns] = build_program(ns)
    return _NC_CACHE[ns]


def _prep_shared(A, SZ):
    """Host tables that are shared across cores (O(K) and O(N*NG))."""
    a = np.asarray(A, dtype=np.float32).reshape(K)
    ae, ao = a[0::2], a[1::2]                    # [KH]
    w2v = ao - 16.0 * ae
    p = np.arange(P)
    w1 = np.zeros((P, P * NCH), np.float16)
    w2 = np.zeros((P, P * NCH), np.float16)
    for c in range(NCH):
        # chunk c accumulates into rows 8*(c%16)+p//16 of its psum tile
        col = P * c + GPC * (c % (NCH // 2)) + p // (GROUP // 2)
        w1[p, col] = ae[P * c + p].astype(np.float16)
        w2[p, col] = w2v[P * c + p].astype(np.float16)
    sag = a.reshape(NG, GROUP).sum(-1)           # [NG] fp32
    s = np.asarray(SZ[:, :, 0], dtype=np.float32)
    z = np.asarray(SZ[:, :, 1], dtype=np.float32)
    zsum = ((z - 8.0 * s) * sag[None, :]).sum(-1)   # [N] fp32
    onesv = np.ones((P, 1), np.float16)
    # unused correction hook (kept for layout flexibility)
    cgm = np.zeros((P, 2), np.float32)
    return w1, w2, onesv, zsum.astype(np.float16), cgm


def _make_in_maps(A, B, SZ):
    B = np.asarray(B)
    SZ = np.asarray(SZ)
    # dense byte view of B (little-endian low half of each int32), biased to int8
    bb = B.view(np.int16)[:, 0::2]
    b8 = bb.astype(np.uint8)                     # [N, KH] bytes 0..255
    w1, w2, onesv, zsum, cgm = _prep_shared(A, SZ)
    in_maps = []
    for cix in range(NCORES):
        r0, r1 = cix * NS, (cix + 1) * NS
        in_maps.append(
            {
                "BT": np.ascontiguousarray(b8[r0:r1].T),                 # [KH, NS]
                "ST": np.ascontiguousarray(SZ[r0:r1, :, 0].T),           # [NG, NS]
                "W1": w1,
                "W2": w2,
                "ONES": onesv,
                "ZS": np.ascontiguousarray(zsum[r0:r1]).reshape(1, NS),
                "CG": cgm,
            }
        )
    return in_maps


def kernel(A, B, scalesAndZeros):
    A = np.asarray(A)
    in_maps = _make_in_maps(A, B, scalesAndZeros)
    nc = _get_program()
    res = run_bass_kernel_spmd(nc, in_maps, core_ids=list(range(NCORES)))
    out = np.concatenate([res.results[c]["OUT"] for c in range(NCORES)])
    return out.reshape(1, N).astype(np.float16)


if __name__ == "__main__":
    rng = np.random.default_rng(0)
    A = rng.standard_normal((M, K)).astype(np.float16)
    B = rng.integers(0, 256, (N, KH)).astype(np.int32)
    SZ = rng.standard_normal((N, NG, 2)).astype(np.float16)
    out = kernel(A, B, SZ)
    print(out.shape, out.dtype, out[0, :8])
